# revision 4
# baseline (speedup 1.0000x reference)
"""Trainium2 kernel for nn_DisjointSet: pointer-chase with compaction.

reference semantics:
    f_conv  = fixed point of f <- f[f]   (root id per point)
    gathered = values[f_conv]

Algorithm (8-core SPMD, core c owns points i with i % 8 == c):
  T16[j] = (father[j], values[j].bits, 0, 0)   16-byte rows, replicated.
  R1 (dense): gather T16[father[i]] for every owned point ->
      (x1, v1) = (father[father[i]], values[father[i]]).
      x1 == father[i]  <=>  depth(i) <= 1: output row = (father[i], v1) final.
      Else append the pair (pos_i, x1) to buffer B2 (stream compaction via
      per-partition-striped scatter).
  Rk (k >= 2): for each active pair (pos, val): gather T16[val] -> (nv, v).
      nv == val  =>  val is the root: Z[pos] = (val, v).   Else append
      (pos, nv) to B(k+1).  Expected actives shrink ~(k+1)!-fold per round;
      max depth of this forest family is ~10-13, rounds go to 13.
  Z and all B regions live in one DRAM tensor so a single scatter per
  partition writes both resolved results and continuations.

DMA primitive (measured on HW): an indirect DMA whose SBUF side is
  [1 partition, W runs, run-of-2, stride 4] emits W descriptors (994 ns +
  0.34 ns/desc SWDGE). Offsets are read column-major across all 128 SBUF
  partitions, so offset streams are pre-transposed in [128,128] squares with
  the DVE transpose. The hardware scales each offset by the run stride (4),
  which is exactly 16-byte-row addressing into T16.

Falls back to the original pointer-doubling kernel (exact, slower) if the
host-side verification of the chase result fails (e.g. freak input deeper
than 13 or beyond compaction capacity).
"""

import time

import numpy as np

import concourse.bass as bass
import concourse.mybir as mybir

P = 128
N_EXPECTED = 16_777_216
N = N_EXPECTED
N_CORES = 8
B = N // N_CORES
C1 = 3072
SUB = 512
SENT = 0x7FF00000
Z_ROWS = B

REGIONS = [
    (2097152, 128, 3072, 128, 3072),  # B2
    (2490368, 128, 1024, 128, 1024),  # B3
    (2621440, 128, 256, 128, 256),    # B4
    (2686976, 64, 128, 128, 64),      # B5
    (2752512, 16, 128, 64, 32),       # B6
    (2818048, 8, 64, 16, 32),         # B7
    (2883584, 2, 128, 8, 32),         # B8
    (2949120, 1, 128, 2, 64),         # B9
    (3014656, 1, 128, 1, 128),        # B10
    (3080192, 1, 128, 1, 128),        # B11
    (3145728, 1, 128, 1, 128),        # B12
    (3211264, 1, 128, 1, 128),        # B13
    (3276800, 1, 128, 1, 128),        # B14 (never read; must stay empty)
]
ZB_ROWS = 3342336
NR = len(REGIONS)


def _enable_dynamic_dma():
    import concourse.bass_utils as bu

    if getattr(bu, "_dyn_dma_patched", False):
        return
    orig = bu.run_command

    def patched(cmd, *a, **kw):
        if cmd and isinstance(cmd[0], str) and "walrus_driver" in cmd[0]:
            if not any(str(c).startswith("--dge-levels") for c in cmd):
                cmd = list(cmd) + [
                    "--dge-levels=io,spill_reload,scalar_dynamic_offset,vector_dynamic_offsets",
                ]
        return orig(cmd, *a, **kw)

    bu.run_command = patched
    bu._dyn_dma_patched = True


# ======================= chase kernel =======================


def _build_chase():
    nc = bass.Bass()
    AL = mybir.AluOpType

    t16 = nc.declare_dram_parameter("t16", [N * 4], mybir.dt.int32, isOutput=False)
    fsl = nc.declare_dram_parameter("fsl", [B], mybir.dt.int32, isOutput=False)
    posa = nc.declare_dram_parameter("posa", [B], mybir.dt.int32, isOutput=False)
    vown = nc.declare_dram_parameter("vown", [B], mybir.dt.int32, isOutput=False)
    ccf = nc.declare_dram_parameter("ccf", [P], mybir.dt.float32, isOutput=False)
    basep = nc.declare_dram_parameter("basep", [P * NR], mybir.dt.float32, isOutput=False)
    limp = nc.declare_dram_parameter("limp", [P * NR], mybir.dt.float32, isOutput=False)
    zb = nc.declare_dram_parameter("zb", [ZB_ROWS * 4], mybir.dt.int32, isOutput=True)

    t16_r = t16.rearrange("(n four) -> n four", four=4)
    zb_r = zb.rearrange("(n four) -> n four", four=4)
    zb_t = zb.rearrange("(t p w four) -> t p w four", p=P, w=SUB, four=4)
    fsl_t = fsl.rearrange("(t p w) -> t p w", p=P, w=SUB)
    vown_t = vown.rearrange("(t p w) -> t p w", p=P, w=SUB)
    pos_t = posa.rearrange("(t p w) -> t p w", p=P, w=SUB)

    GP, SY, DV = mybir.EngineType.Pool, mybir.EngineType.SP, mybir.EngineType.DVE

    import contextlib
    with contextlib.ExitStack() as _st:
        _e = _st.enter_context
        _e(nc.allow_non_contiguous_dma(reason="per-element gather/scatter"))
        G4 = _e(nc.sbuf_tensor("G4", [P, C1, 4], mybir.dt.int32))
        PT = _e(nc.sbuf_tensor("PT", [P, C1, 4], mybir.dt.int32))
        OT = _e(nc.sbuf_tensor("OT", [P, C1], mybir.dt.int32))
        VF = _e(nc.sbuf_tensor("VF", [P, C1], mybir.dt.int32))
        PM = _e(nc.sbuf_tensor("PM", [P, C1], mybir.dt.int32))
        DI = _e(nc.sbuf_tensor("DI", [P, C1], mybir.dt.int32))
        FA = _e(nc.sbuf_tensor("FA", [P, C1], mybir.dt.float32))
        FB = _e(nc.sbuf_tensor("FB", [P, C1], mybir.dt.float32))
        FC = _e(nc.sbuf_tensor("FC", [P, C1], mybir.dt.float32))
        FD = _e(nc.sbuf_tensor("FD", [P, C1], mybir.dt.float32))
        VO = _e(nc.sbuf_tensor("VO", [P, C1], mybir.dt.int32))
        CC = _e(nc.sbuf_tensor("CC", [P, 1], mybir.dt.float32))
        BASE = _e(nc.sbuf_tensor("BASE", [P, NR], mybir.dt.float32))
        LIM = _e(nc.sbuf_tensor("LIM", [P, NR], mybir.dt.float32))
        s_ld = _e(nc.semaphore("s_ld"))
        s_g = _e(nc.semaphore("s_g"))
        s_sc = _e(nc.semaphore("s_sc"))
        s_st = _e(nc.semaphore("s_st"))
        s_v = _e(nc.semaphore("s_v"))
        c = {"ld": 0, "g": 0, "sc": 0, "st": 0, "v": 0}
        breg_n = nc.gpsimd.to_reg(N - 1)
        breg_zb = nc.gpsimd.to_reg(ZB_ROWS - 1)

        def ld(dst, src):
            nc.sync.dma_start(out=dst, in_=src).then_inc(s_ld, 16)
            c["ld"] += 16

        def store(dst, src):
            nc.sync.dma_start(out=dst, in_=src).then_inc(s_st, 16)
            c["st"] += 16

        def vmark(i):
            i.then_inc(s_v, 1)
            c["v"] += 1

        def transpose_blocks(dst, src, cols):
            # dst[j%128, p*wc + j//128] = src[p, j] for wc = cols//128.
            # nc.vector.transpose is 32x32-block-local, so compose the full
            # [128,128] transpose from 16 block calls with swapped grid
            # coordinates.
            wc = max(1, cols // 128)
            nsq = wc if cols > 128 else 1
            for d in range(nsq):
                base = d * 128
                for bi in range(4):
                    for bj in range(4):
                        nc.vector.transpose(
                            dst[
                                32 * bj : 32 * bj + 32,
                                (32 * bi) * wc + d : (32 * bi + 31) * wc + d + 1 : wc,
                            ],
                            src[
                                32 * bi : 32 * bi + 32,
                                base + 32 * bj : base + 32 * bj + 32,
                            ],
                        )

        # ---- init: base/limit tables; SENT-prefill of B regions ----
        ld(BASE[:], basep.rearrange("(p r) -> p r", p=P))
        ld(CC[:], ccf.rearrange("(p one) -> p one", one=1))
        ld(LIM[:], limp.rearrange("(p r) -> p r", p=P))
        nc.vector.memset(OT[:], SENT)
        vmark(nc.vector.tensor_copy(OT[:, 0:1], OT[:, 0:1]))
        nc.sync.wait_ge(s_v, c["v"])
        sent4 = OT[:, 0:2048].rearrange("p (w four) -> p w four", four=4)
        for org, rnp, rw, wnp, ws in REGIONS:
            true_rows = rnp * rw
            done = 0
            while done < true_rows:
                k_rows = min(true_rows - done, 65536)
                kw = k_rows // P
                dst = zb_r[bass.ds(org + done, k_rows)].rearrange(
                    "(p w) four -> p w four", w=kw
                )
                store(dst, sent4[:, 0:kw, :])
                done += k_rows
        nc.gpsimd.wait_ge(s_ld, c["ld"])
        nc.vector.wait_ge(s_ld, c["ld"])
        nc.sync.wait_ge(s_st, c["st"])
        nc.multi_engine_barrier([GP, SY, DV])

        def gathers(np_, w_):
            wc = max(1, w_ // 128)
            for p in range(np_):
                nc.gpsimd.indirect_dma_start(
                    out=G4[p : p + 1, 0:w_, 0:2],
                    out_offset=None,
                    in_=t16_r,
                    in_offset=bass.IndirectOffsetOnAxis(
                        ap=OT[:, p * wc : (p + 1) * wc], axis=0
                    ),
                    bounds_check=breg_n,
                    oob_is_err=False,
                ).then_inc(s_g, 16)
                c["g"] += 16

        def scatters(np_, w_):
            wc = max(1, w_ // 128)
            for p in range(np_):
                nc.gpsimd.indirect_dma_start(
                    out=zb_r,
                    out_offset=bass.IndirectOffsetOnAxis(
                        ap=OT[:, p * wc : (p + 1) * wc], axis=0
                    ),
                    in_=G4[p : p + 1, 0:w_, 2:4],
                    in_offset=None,
                    bounds_check=breg_zb,
                    oob_is_err=False,
                ).then_inc(s_sc, 16)
                c["sc"] += 16

        # ================= R1: dense over owned points =================
        n_sub = B // 65536  # 32 subtiles of [128, 512]
        sub = 0
        while sub < n_sub:
            k = min(6, n_sub - sub)
            w_ = k * SUB
            for j in range(k):
                ld(VF[:, j * SUB : (j + 1) * SUB], fsl_t[bass.ds(sub + j, 1)][0])
                ld(PM[:, j * SUB : (j + 1) * SUB], pos_t[bass.ds(sub + j, 1)][0])
                ld(VO[:, j * SUB : (j + 1) * SUB], vown_t[bass.ds(sub + j, 1)][0])
            nc.vector.wait_ge(s_ld, c["ld"])
            # root-skip: offsets = father, or SENT where father == gidx
            nc.vector.tensor_copy(FA[:, 0:w_], PM[:, 0:w_])
            nc.vector.tensor_scalar(
                out=FA[:, 0:w_], in0=FA[:, 0:w_], scalar1=8.0, scalar2=CC[:, 0:1],
                op0=AL.mult, op1=AL.add,
            )
            nc.vector.tensor_copy(FB[:, 0:w_], VF[:, 0:w_])
            nc.vector.tensor_tensor(out=FD[:, 0:w_], in0=FA[:, 0:w_], in1=FB[:, 0:w_], op=AL.is_equal)
            nc.vector.tensor_scalar(
                out=FA[:, 0:w_], in0=FD[:, 0:w_], scalar1=-1.0, scalar2=1.0,
                op0=AL.mult, op1=AL.add,
            )
            nc.vector.tensor_tensor(out=FA[:, 0:w_], in0=FB[:, 0:w_], in1=FA[:, 0:w_], op=AL.mult)
            nc.vector.tensor_scalar(
                out=FC[:, 0:w_], in0=FD[:, 0:w_], scalar1=float(SENT), scalar2=None,
                op0=AL.mult,
            )
            nc.vector.tensor_tensor(out=FA[:, 0:w_], in0=FA[:, 0:w_], in1=FC[:, 0:w_], op=AL.add)
            nc.vector.tensor_copy(DI[:, 0:w_], FA[:, 0:w_])
            vmark(transpose_blocks(OT, DI, w_) or nc.vector.tensor_copy(OT[:, 0:1], OT[:, 0:1]))
            nc.gpsimd.wait_ge(s_v, c["v"])
            gathers(P, w_)
            nc.vector.wait_ge(s_g, c["g"])
            # payload lane2 early; then PM becomes the int root mask
            nc.vector.tensor_copy(G4[:, 0:w_, 2:3], PM[:, 0:w_])
            nc.vector.tensor_copy(PM[:, 0:w_], FD[:, 0:w_])
            # root lanes: lane0 := father (= own id = root), lane1 := own value
            nc.vector.copy_predicated(G4[:, 0:w_, 0], PM[:, 0:w_], VF[:, 0:w_])
            nc.vector.copy_predicated(G4[:, 0:w_, 1], PM[:, 0:w_], VO[:, 0:w_])
            # fp32 chain; cont = (x1 != father) * notroot
            nc.vector.tensor_copy(FA[:, 0:w_], G4[:, 0:w_, 0])
            nc.vector.tensor_tensor(out=FC[:, 0:w_], in0=FA[:, 0:w_], in1=FB[:, 0:w_], op=AL.not_equal)
            nc.vector.tensor_scalar(
                out=FA[:, 0:w_], in0=FD[:, 0:w_], scalar1=-1.0, scalar2=1.0,
                op0=AL.mult, op1=AL.add,
            )
            nc.vector.tensor_tensor(out=FC[:, 0:w_], in0=FC[:, 0:w_], in1=FA[:, 0:w_], op=AL.mult)
            nc.vector.tensor_tensor_scan(
                out=FA[:, 0:w_], data0=FC[:, 0:w_], data1=FC[:, 0:w_],
                initial=0.0, op0=AL.add, op1=AL.max,
            )
            nc.vector.scalar_tensor_tensor(
                out=FD[:, 0:w_], in0=FA[:, 0:w_], scalar=BASE[:, 0:1],
                in1=FC[:, 0:w_], op0=AL.add, op1=AL.subtract,
            )
            nc.vector.tensor_tensor(
                out=BASE[:, 0:1], in0=BASE[:, 0:1], in1=FA[:, w_ - 1 : w_], op=AL.add
            )
            nc.vector.scalar_tensor_tensor(
                out=FC[:, 0:w_], in0=FD[:, 0:w_], scalar=LIM[:, 0:1],
                in1=FC[:, 0:w_], op0=AL.is_lt, op1=AL.mult,
            )
            nc.vector.tensor_scalar(
                out=FA[:, 0:w_], in0=FC[:, 0:w_], scalar1=float(-SENT),
                scalar2=float(SENT), op0=AL.mult, op1=AL.add,
            )
            nc.vector.tensor_tensor(out=FD[:, 0:w_], in0=FD[:, 0:w_], in1=FC[:, 0:w_], op=AL.mult)
            nc.vector.tensor_tensor(out=FD[:, 0:w_], in0=FD[:, 0:w_], in1=FA[:, 0:w_], op=AL.add)
            nc.vector.tensor_copy(DI[:, 0:w_], FD[:, 0:w_])
            nc.vector.tensor_copy(G4[:, 0:w_, 3:4], G4[:, 0:w_, 0:1])
            vmark(transpose_blocks(OT, DI, w_) or nc.vector.tensor_copy(OT[:, 0:1], OT[:, 0:1]))
            nc.gpsimd.wait_ge(s_v, c["v"])
            nc.sync.wait_ge(s_v, c["v"])
            for j in range(k):
                store(zb_t[bass.ds(sub + j, 1)][0], G4[:, j * SUB : (j + 1) * SUB, :])
            scatters(P, w_)
            nc.sync.wait_ge(s_sc, c["sc"])
            nc.sync.wait_ge(s_st, c["st"])
            nc.gpsimd.wait_ge(s_st, c["st"])
            nc.gpsimd.wait_ge(s_sc, c["sc"])
            nc.multi_engine_barrier([GP, SY, DV])
            sub += k

        # ================= chase rounds =================
        for ri in range(NR - 1):
            org, rnp, rw, wnp, ws = REGIONS[ri]
            bcol = ri + 1
            true_rows = rnp * rw
            if true_rows >= 65536:
                nsubr = true_rows // 65536
                for j in range(nsubr):
                    ld(PT[:, j * SUB : (j + 1) * SUB, :], zb_t[bass.ds(org // 65536 + j, 1)][0])
                w_, np_ = nsubr * SUB, P
            else:
                src = zb_r[bass.ds(org, true_rows)].rearrange(
                    "(p w) four -> p w four", w=rw
                )
                ld(PT[0:rnp, 0:rw, :], src)
                w_, np_ = rw, rnp
            nc.vector.wait_ge(s_ld, c["ld"])
            nc.vector.tensor_copy(VF[0:P, 0:w_], PT[0:P, 0:w_, 1])
            vmark(transpose_blocks(OT, VF, w_) or nc.vector.tensor_copy(OT[:, 0:1], OT[:, 0:1]))
            nc.gpsimd.wait_ge(s_v, c["v"])
            gathers(np_, w_)
            nc.vector.wait_ge(s_g, c["g"])
            nc.vector.tensor_copy(FA[:, 0:w_], PT[:, 0:w_, 1])
            nc.vector.tensor_copy(FB[:, 0:w_], G4[:, 0:w_, 0])
            nc.vector.tensor_scalar(
                out=FC[:, 0:w_], in0=FA[:, 0:w_], scalar1=float(N),
                scalar2=None, op0=AL.is_lt,
            )
            nc.vector.tensor_tensor(out=FD[:, 0:w_], in0=FB[:, 0:w_], in1=FA[:, 0:w_], op=AL.is_equal)
            nc.vector.tensor_tensor(out=FD[:, 0:w_], in0=FD[:, 0:w_], in1=FC[:, 0:w_], op=AL.mult)
            nc.vector.tensor_copy(PM[:, 0:w_], FD[:, 0:w_])
            nc.vector.tensor_tensor(out=FC[:, 0:w_], in0=FC[:, 0:w_], in1=FD[:, 0:w_], op=AL.subtract)
            nc.vector.tensor_tensor_scan(
                out=FA[:, 0:w_], data0=FC[:, 0:w_], data1=FC[:, 0:w_],
                initial=0.0, op0=AL.add, op1=AL.max,
            )
            nc.vector.scalar_tensor_tensor(
                out=FB[:, 0:w_], in0=FA[:, 0:w_], scalar=BASE[:, bcol : bcol + 1],
                in1=FC[:, 0:w_], op0=AL.add, op1=AL.subtract,
            )
            nc.vector.tensor_tensor(
                out=BASE[:, bcol : bcol + 1], in0=BASE[:, bcol : bcol + 1],
                in1=FA[:, w_ - 1 : w_], op=AL.add,
            )
            nc.vector.scalar_tensor_tensor(
                out=FC[:, 0:w_], in0=FB[:, 0:w_], scalar=LIM[:, bcol : bcol + 1],
                in1=FC[:, 0:w_], op0=AL.is_lt, op1=AL.mult,
            )
            nc.vector.tensor_scalar(
                out=FA[:, 0:w_], in0=FC[:, 0:w_], scalar1=float(-SENT),
                scalar2=float(SENT), op0=AL.mult, op1=AL.add,
            )
            nc.vector.tensor_tensor(out=FB[:, 0:w_], in0=FB[:, 0:w_], in1=FC[:, 0:w_], op=AL.mult)
            nc.vector.tensor_tensor(out=FB[:, 0:w_], in0=FB[:, 0:w_], in1=FA[:, 0:w_], op=AL.add)
            nc.vector.tensor_scalar(
                out=FA[:, 0:w_], in0=FD[:, 0:w_], scalar1=-1.0, scalar2=1.0,
                op0=AL.mult, op1=AL.add,
            )
            nc.vector.tensor_tensor(out=FB[:, 0:w_], in0=FB[:, 0:w_], in1=FA[:, 0:w_], op=AL.mult)
            nc.vector.tensor_copy(FA[:, 0:w_], PT[:, 0:w_, 0])
            nc.vector.tensor_tensor(out=FA[:, 0:w_], in0=FA[:, 0:w_], in1=FD[:, 0:w_], op=AL.mult)
            nc.vector.tensor_tensor(out=FB[:, 0:w_], in0=FB[:, 0:w_], in1=FA[:, 0:w_], op=AL.add)
            nc.vector.tensor_copy(DI[:, 0:w_], FB[:, 0:w_])
            nc.vector.tensor_copy(G4[:, 0:w_, 2:3], PT[:, 0:w_, 0:1])
            nc.vector.copy_predicated(G4[:, 0:w_, 2], PM[:, 0:w_], PT[:, 0:w_, 1])
            nc.vector.tensor_copy(G4[:, 0:w_, 3:4], G4[:, 0:w_, 0:1])
            nc.vector.copy_predicated(G4[:, 0:w_, 3], PM[:, 0:w_], G4[:, 0:w_, 1])
            vmark(transpose_blocks(OT, DI, w_) or nc.vector.tensor_copy(OT[:, 0:1], OT[:, 0:1]))
            nc.gpsimd.wait_ge(s_v, c["v"])
            scatters(np_, w_)
            nc.gpsimd.wait_ge(s_sc, c["sc"])
            nc.sync.wait_ge(s_sc, c["sc"])
            nc.multi_engine_barrier([GP, SY, DV])

    return nc


def _chase_inputs(father_i32, values_f32):
    t16 = np.zeros((N, 4), dtype=np.int32)
    t16[:, 0] = father_i32
    t16[:, 1] = values_f32.view(np.int32)
    t16 = np.ascontiguousarray(t16.reshape(-1))
    pos = np.arange(B, dtype=np.int32)
    bases = np.zeros((P, NR), dtype=np.float32)
    lims = np.zeros((P, NR), dtype=np.float32)
    for ri, (org, rnp, rw, wnp, ws) in enumerate(REGIONS):
        for p in range(P):
            if p < wnp:
                bases[p, ri] = org + p * ws
                lims[p, ri] = org + (p + 1) * ws
            else:
                bases[p, ri] = org
                lims[p, ri] = org
    in_maps = []
    for cc in range(N_CORES):
        in_maps.append(
            {
                "t16": t16,
                "fsl": np.ascontiguousarray(father_i32[cc::N_CORES]),
                "vown": np.ascontiguousarray(values_f32[cc::N_CORES]).view(np.int32),
                "ccf": np.full(P, float(cc), dtype=np.float32),
                "posa": pos,
                "basep": bases.reshape(-1),
                "limp": lims.reshape(-1),
            }
        )
    return in_maps


def _run_chase(father_i32, values_f32, trace=False):
    from concourse.bass_utils import run_bass_kernel_spmd

    _enable_dynamic_dma()
    in_maps = _chase_inputs(father_i32, values_f32)
    nc = _build_chase()
    res = run_bass_kernel_spmd(nc, in_maps, list(range(N_CORES)), trace=trace)
    froot = np.empty(N, dtype=np.int32)
    gath = np.empty(N, dtype=np.float32)
    for cc in range(N_CORES):
        zbo = res.results[cc]["zb"].reshape(ZB_ROWS, 4)
        froot[cc::N_CORES] = zbo[:B, 0]
        gath[cc::N_CORES] = zbo[:B, 1].view(np.float32)
    return froot, gath, res


# ================= fallback: pointer-doubling (original) =================

DEFAULT_JUMPS = 4


def _build_doubling(Nn, Bb, W, n_jumps):
    assert Bb % (P * W) == 0
    NT = Bb // (P * W)
    nc = bass.Bass()

    father = nc.declare_dram_parameter("father", [Nn], mybir.dt.int32, isOutput=False)
    values = nc.declare_dram_parameter("values", [Nn], mybir.dt.float32, isOutput=False)
    fslice = nc.declare_dram_parameter("fslice", [Bb], mybir.dt.int32, isOutput=False)
    froot_out = nc.declare_dram_parameter("froot", [Bb], mybir.dt.int32, isOutput=True)
    gath_out = nc.declare_dram_parameter("gathered", [Bb], mybir.dt.float32, isOutput=True)

    x_dram = nc.dram_tensor("x_dram", [Bb], mybir.dt.int32)
    f_shared = nc.dram_tensor("f_shared", [Nn], mybir.dt.int32, addr_space="Shared")
    f_full = nc.dram_tensor("f_full", [Nn], mybir.dt.int32)

    father_rows = father.rearrange("(n one) -> n one", one=1)
    ffull_rows = f_full.rearrange("(n one) -> n one", one=1)
    values_rows = values.rearrange("(n one) -> n one", one=1)

    fslice_t = fslice.rearrange("(t p w) -> t p w", p=P, w=W)
    x_t = x_dram.rearrange("(t p w) -> t p w", p=P, w=W)
    froot_t = froot_out.rearrange("(t p w) -> t p w", p=P, w=W)
    gath_t = gath_out.rearrange("(t p w) -> t p w", p=P, w=W)

    core_ids = list(range(N_CORES))

    with (
        nc.sbuf_tensor("offs_sb", [P, W], mybir.dt.int32) as offs_sb,
        nc.sbuf_tensor("g_sb", [P, W], mybir.dt.int32) as g_sb,
        nc.sbuf_tensor("gf_sb", [P, W], mybir.dt.float32) as gf_sb,
        nc.semaphore("s_ld") as s_ld,
        nc.semaphore("s_g") as s_g,
        nc.semaphore("s_st") as s_st,
        nc.semaphore("s_cp") as s_cp,
        nc.semaphore("cc_sem") as cc_sem,
    ):
        GP, SY = mybir.EngineType.Pool, mybir.EngineType.SP

        r_gp = nc.gpsimd.alloc_register("thr_gp")
        r_gp2 = nc.gpsimd.alloc_register("thr_gp2")
        r_sy = nc.sync.alloc_register("thr_sy")
        r_syg = nc.sync.alloc_register("thr_syg")
        r_sy2 = nc.sync.alloc_register("thr_sy2")

        state = {"iters": 0, "cc": 0, "cp": 0}

        def gather_pass(src_tiled, table_rows, out_sb, stores, n_st):
            base = state["iters"]
            with nc.Fori(0, NT, engines=[GP, SY]) as it:
                nc.gpsimd.reg_alu(r_gp, it, base + 1, op=mybir.AluOpType.add)
                nc.gpsimd.reg_alu(r_gp, r_gp, 16, op=mybir.AluOpType.mult)
                nc.gpsimd.reg_alu(r_gp2, r_gp, W, op=mybir.AluOpType.mult)
                nc.sync.reg_alu(r_sy, it, base + 1, op=mybir.AluOpType.add)
                nc.sync.reg_alu(r_sy, r_sy, 16, op=mybir.AluOpType.mult)
                nc.sync.reg_alu(r_syg, r_sy, W, op=mybir.AluOpType.mult)

                nc.sync.dma_start(
                    out=offs_sb[:], in_=src_tiled[bass.ds(it, 1)][0]
                ).then_inc(s_ld, 16)
                nc.gpsimd.wait_ge(s_ld, nc.gpsimd.snap(r_gp))
                for w in range(W):
                    nc.gpsimd.indirect_dma_start(
                        out=out_sb[:, w:w + 1],
                        out_offset=None,
                        in_=table_rows,
                        in_offset=bass.IndirectOffsetOnAxis(
                            ap=offs_sb[:, w:w + 1], axis=0
                        ),
                    ).then_inc(s_g, 16)
                nc.sync.wait_ge(s_g, nc.sync.snap(r_syg))
                stores(it)
                if n_st == 1:
                    nc.sync.wait_ge(s_st, nc.sync.snap(r_sy))
                else:
                    nc.sync.reg_alu(r_sy2, it, 1, op=mybir.AluOpType.add)
                    nc.sync.reg_alu(r_sy2, r_sy2, 16 * n_st, op=mybir.AluOpType.mult)
                    nc.sync.reg_alu(
                        r_sy2, r_sy2, 16 * state["st_prior"], op=mybir.AluOpType.add
                    )
                    nc.sync.wait_ge(s_st, nc.sync.snap(r_sy2))
                nc.gpsimd.wait_ge(s_g, nc.gpsimd.snap(r_gp2))
                nc.multi_engine_barrier([GP, SY])
            state["iters"] += NT

        def st_jump(it):
            nc.sync.dma_start(
                out=x_t[bass.ds(it, 1)][0], in_=g_sb[:]
            ).then_inc(s_st, 16)

        gather_pass(fslice_t, father_rows, g_sb, st_jump, 1)

        for _ in range(1, n_jumps):
            nc.gpsimd.collective_compute(
                "AllGather",
                mybir.AluOpType.bypass,
                replica_groups=[core_ids],
                ins=[x_dram[:]],
                outs=[f_shared[:]],
            ).then_inc(cc_sem, 1)
            state["cc"] += 1
            nc.sync.wait_ge(cc_sem, state["cc"])
            nc.sync.dma_start(out=f_full[:], in_=f_shared[:]).then_inc(s_cp, 16)
            state["cp"] += 16
            nc.sync.wait_ge(s_cp, state["cp"])
            nc.multi_engine_barrier([GP, SY])
            gather_pass(x_t, ffull_rows, g_sb, st_jump, 1)

        state["st_prior"] = state["iters"]

        def st_vg(it):
            nc.sync.dma_start(
                out=gath_t[bass.ds(it, 1)][0], in_=gf_sb[:]
            ).then_inc(s_st, 16)
            nc.sync.dma_start(
                out=froot_t[bass.ds(it, 1)][0], in_=offs_sb[:]
            ).then_inc(s_st, 16)

        gather_pass(x_t, values_rows, gf_sb, st_vg, 2)

    return nc


def _run_doubling(father_i32, values_f32, n_jumps, trace=False):
    from concourse.bass_utils import run_bass_kernel_spmd

    _enable_dynamic_dma()
    Nn = len(father_i32)
    Bb = Nn // N_CORES
    W = 512
    in_maps = [
        {
            "father": father_i32,
            "values": values_f32,
            "fslice": father_i32[cc * Bb:(cc + 1) * Bb],
        }
        for cc in range(N_CORES)
    ]
    nc = _build_doubling(Nn, Bb, W, n_jumps)
    res = run_bass_kernel_spmd(nc, in_maps, list(range(N_CORES)), trace=trace)
    froot = np.concatenate([res.results[cc]["froot"] for cc in range(N_CORES)])
    gath = np.concatenate([res.results[cc]["gathered"] for cc in range(N_CORES)])
    return froot, gath, res


# ================= verification + entry =================


def _verified(father_i32, values_f32, froot, gath):
    idx = np.arange(len(father_i32), dtype=np.int64)
    fr = froot.astype(np.int64)
    fa = father_i32.astype(np.int64)
    if fr.min() < 0 or fr.max() >= len(father_i32):
        return False
    if not np.array_equal(fr == idx, fa == idx):
        return False
    if not np.array_equal(fr[fr], fr):
        return False
    if not np.array_equal(fr[fa], fr):
        return False
    if not np.array_equal(gath, values_f32[fr]):
        return False
    return True


def kernel(father: np.ndarray, values: np.ndarray, _trace=False):
    assert father.shape == (N_EXPECTED,) and values.shape == (N_EXPECTED,), (
        father.shape,
        values.shape,
    )
    out_dtype = father.dtype
    father_i32 = np.ascontiguousarray(father.astype(np.int32))
    values_f32 = np.ascontiguousarray(values.astype(np.float32))

    last_err = None
    for attempt in range(2):
        try:
            froot, gath, res = _run_chase(father_i32, values_f32, trace=_trace)
            if _verified(father_i32, values_f32, froot, gath):
                kernel.last_result = res
                return froot.astype(out_dtype), gath
        except Exception as e:  # transient device wedge: retry once
            last_err = e
            time.sleep(10)

    # fallback: exact pointer-doubling kernel (slow but proven)
    n_jumps = DEFAULT_JUMPS
    for _ in range(3):
        try:
            froot, gath, res = _run_doubling(father_i32, values_f32, n_jumps, trace=_trace)
        except Exception as e:
            last_err = e
            time.sleep(10)
            continue
        if _verified(father_i32, values_f32, froot, gath):
            kernel.last_result = res
            return froot.astype(out_dtype), gath
        n_jumps += 2
    raise last_err if last_err else RuntimeError("kernel failed verification")
